# revision 27
# baseline (speedup 1.0000x reference)
"""GhostAttention (B=2, T=2048, C=2048, H=16) on 8 Trainium2 NeuronCores.

Sharding: tensor-parallel over heads (Megatron-style). Core c owns heads
{2c, 2c+1}: it gets the 256 matching rows of Wq/Wk/Wv (column-parallel) and
the 256 matching columns of Wo (row-parallel), computes QKV projections,
masked-relu attention and its partial output projection for both batches,
and writes a full-shape partial y. The host sums the 8 partials.

Per-core dataflow (all matmuls fp32r: fp32 storage, bf16-rate on the PE):
  phase 1: q,k in (hd, tok) layout and v in (tok, hd) layout, accumulating
           over 16 K-tiles of x^T streamed from HBM; PSUM drains split
           ACT/DVE so the next tile's matmuls unblock sooner.
  phase 2: S^T blocks (tk=128, tq=512) = k-stationary @ q-moving, emitted
           one block ahead of the AV consumer (pps bufs=2) so the relu
           drain overlaps PE work; diagonal blocks are windowed to the
           unmasked column range and get a causal 0/1 mask multiply on DVE;
           AV accumulates v-stationary @ w-moving into (hd, tq) PSUM; a
           ones-column matmul accumulates the normalizer; the tail
           (EPS-add, fast reciprocal, partition-broadcast, po*rec multiply)
           runs entirely on DVE+GpSimd so the PE rolls straight into the
           next head's score blocks.
  phase 3: out-projection, attn-stationary @ Wo-moving -> (tok, o) PSUM,
           drained alternately on ACT/DVE to SBUF and DMA'd to the partial
           output.
Weights are DMA'd per k-tile so the first projection matmul only waits for
its own slice, not the whole weight load.
"""

import math
import sys

if "/opt/trn_rl_repo" not in sys.path:
    sys.path.insert(0, "/opt/trn_rl_repo")

import numpy as np
from contextlib import ExitStack

import concourse.bass as bass
import concourse.mybir as mybir
import concourse.tile as tile
from concourse.bass import ts, ds
from concourse.bass_utils import run_bass_kernel_spmd
from concourse.vector_clock import ScopedClock, VectorClock


def _split_drain_and_barrier(self, tick_clock, wait_clock):
    # This image's walrus caps sem waits per instruction; split the Tile-tail
    # drain waits across single-wait SP nops instead.
    gc = tick_clock.global_clock
    n = len(gc)
    for proc in range(n):
        t = gc[proc]
        if t <= 0:
            continue
        vc = VectorClock([0] * n)
        vc.require_at_least(proc, t)
        nop_inst = self.nc.sync.nop()
        wait_clock.add_sem_waits(nop_inst.ins, ScopedClock({None: vc}))
    self.nc.sync.drain()
    self.nc.all_engine_barrier()
    assert self.sems is not None
    popped = self.nc._tile_sem_poison_stack.pop()
    assert popped is self._sem_poison
    self.nc.clear_and_free_semaphores(list(self.sems.allocated().values()))
    self.nc.all_engine_barrier()


tile.TileContext._drain_and_barrier = _split_drain_and_barrier

_ws_counter = [0]


def split_excess_waits(nc, max_waits=1):
    """Hoist extra per-instruction sem waits onto preceding same-engine NoOps
    (same queue => they execute, and therefore wait, before the instruction)."""
    for fn in nc.m.functions:
        for blk in fn.blocks:
            insts = list(blk.instructions)
            out = []
            changed = False
            for inst in insts:
                si = inst.sync_info
                if si is not None and si.on_wait and len(si.on_wait) > max_waits:
                    waits = list(si.on_wait)
                    extra, keep = waits[:-max_waits], waits[-max_waits:]
                    for s in range(0, len(extra), max_waits):
                        chunk = extra[s : s + max_waits]
                        _ws_counter[0] += 1
                        nop = mybir.InstNoOp(
                            name=f"I-ws-{_ws_counter[0]}",
                            engine=inst.engine,
                            ins=[],
                            outs=[],
                            sync_info=mybir.SyncInfo(on_wait=chunk, on_update=[]),
                        )
                        out.append(nop)
                    inst.sync_info = mybir.SyncInfo(
                        on_wait=keep, on_update=list(si.on_update)
                    )
                    changed = True
                out.append(inst)
            if changed:
                try:
                    blk.instructions[:] = out
                except Exception:
                    blk.set_instructions(out)
    return nc


B, T, C = 2, 2048, 2048
H = 16
HD = C // H  # 128
N_CORES = 8
H_PER_CORE = H // N_CORES  # 2
CH = HD * H_PER_CORE  # 256 channels per core
SCALE = 1.0 / math.sqrt(HD)
ATTN_BIAS = 0.1  # relu(scores - (-0.1)) = relu(scores + 0.1)
EPS = 1e-6

F32 = mybir.dt.float32
F32R = mybir.dt.float32r
F16 = mybir.dt.float16
AF = mybir.ActivationFunctionType

_NC_CACHE = None


def _build(split_waits=True):
    nc = bass.Bass("TRN2", debug=False)
    xT = nc.dram_tensor("xT", [C, B * T], F32R, kind="ExternalInput")
    wq = nc.dram_tensor("wq", [C, CH], F32R, kind="ExternalInput")
    wk = nc.dram_tensor("wk", [C, CH], F32R, kind="ExternalInput")
    wv = nc.dram_tensor("wv", [C, CH], F32R, kind="ExternalInput")
    wo = nc.dram_tensor("wo", [CH, C], F32R, kind="ExternalInput")
    masks = nc.dram_tensor("masks", [4, 128, 512], F32, kind="ExternalInput")
    y = nc.dram_tensor("y", [B * T, C], F16, kind="ExternalOutput")

    KT = C // 128  # 16 contraction tiles
    NT = T // 512  # 4 query tiles of 512 per batch

    with tile.TileContext(nc) as tc, ExitStack() as ctx:
        consts = ctx.enter_context(tc.tile_pool(name="consts", bufs=1))
        qkvp = ctx.enter_context(tc.tile_pool(name="qkv", bufs=1))
        xinp = ctx.enter_context(tc.tile_pool(name="xin", bufs=8))
        wp = ctx.enter_context(tc.tile_pool(name="wtile", bufs=6))
        attnp = ctx.enter_context(tc.tile_pool(name="attn", bufs=2))
        ystp = ctx.enter_context(tc.tile_pool(name="yst", bufs=2))
        smallp = ctx.enter_context(tc.tile_pool(name="small", bufs=2))

        # one tile per k-slice; the DMAs are emitted interleaved with the
        # first projection pass (one k-tile ahead) so the first matmul waits
        # only for its own slice, not the whole 6 MB weight load
        wq_re = wq.ap().rearrange("(k p) o -> p k o", p=128)
        wk_re = wk.ap().rearrange("(k p) o -> p k o", p=128)
        wv_re = wv.ap().rearrange("(k p) o -> p k o", p=128)
        wq_t, wk_t, wv_t = [], [], []
        for kk in range(KT):
            for lst, nm in ((wq_t, "wq"), (wk_t, "wk"), (wv_t, "wv")):
                lst.append(
                    consts.tile([128, CH], F32R, name=f"{nm}_sb{kk}", tag=f"{nm}{kk}")
                )

        def load_w_slice(kk):
            # issue weight loads from the ACT hwdge queue: the Sync engine's
            # ~0.7us per-DMA issue cost would otherwise throttle the
            # interleaved weight+x stream during the first projection pass
            nc.scalar.dma_start(wq_t[kk][:], wq_re[:, kk, :])
            nc.scalar.dma_start(wk_t[kk][:], wk_re[:, kk, :])
            nc.scalar.dma_start(wv_t[kk][:], wv_re[:, kk, :])

        load_w_slice(0)
        wo_sb = consts.tile([128, H_PER_CORE, C], F32R, name="wo_sb", tag="wo")
        wo_re = wo.ap().rearrange("(h p) o -> p h o", p=128)
        mask_sb = consts.tile([128, 4, 512], F32, name="mask_sb", tag="masks")

        def load_wo_masks():
            for h in range(H_PER_CORE):
                nc.scalar.dma_start(wo_sb[:, h, :], wo_re[:, h, :])
            for r in range(4):
                nc.scalar.dma_start(mask_sb[:, r, :], masks.ap()[r])
        ones_sq_f = consts.tile([128, 128], F32, name="ones_sq_f", tag="ones_sq_f")
        nc.vector.memset(ones_sq_f[:], 1.0)
        ones_sq = consts.tile([128, 128], F32R, name="ones_sq", tag="ones_sq")
        nc.scalar.copy(ones_sq[:], ones_sq_f[:])
        bias_sb = consts.tile([128, 1], F32, name="bias_sb", tag="bias")
        nc.vector.memset(bias_sb[:], ATTN_BIAS)
        eps_sb = consts.tile([128, 1], F32, name="eps_sb", tag="eps")
        nc.vector.memset(eps_sb[:], EPS)

        xT_re = xT.ap().rearrange("(k p) t -> p k t", p=128)  # (128, KT, B*T)

        for b in range(B):
            q_sb = qkvp.tile([128, H_PER_CORE, T], F32R, name="q_sb", tag="q")
            k_sb = qkvp.tile([128, H_PER_CORE, T], F32R, name="k_sb", tag="k")
            v_sb = qkvp.tile([128, T // 128, CH], F32R, name="v_sb", tag="v")

            # ---- phase 1: projections for this batch's 2048 tokens ----
            with tc.tile_pool(name="ps1", bufs=1, space="PSUM") as pp1:
                for n in range(NT):
                    ps_q = [
                        pp1.tile([128, 512], F32, name=f"ps_q{h}", tag=f"psq{h}")
                        for h in (0, 1)
                    ]
                    ps_k = [
                        pp1.tile([128, 512], F32, name=f"ps_k{h}", tag=f"psk{h}")
                        for h in (0, 1)
                    ]
                    ps_v = [
                        pp1.tile([128, 256], F32, name=f"ps_v{s}", tag=f"psv{s}")
                        for s in range(4)
                    ]
                    for kk in range(KT):
                        if b == 0 and n == 0 and kk + 1 < KT:
                            load_w_slice(kk + 1)
                        xin = xinp.tile([128, 512], F32R, name="xin", tag="xin")
                        nc.sync.dma_start(
                            xin[:], xT_re[:, kk, ds(T * b + 512 * n, 512)]
                        )
                        st, sp = kk == 0, kk == KT - 1
                        for h in (0, 1):
                            nc.tensor.matmul(
                                ps_q[h][:],
                                wq_t[kk][:, ts(h, 128)],
                                xin[:],
                                start=st,
                                stop=sp,
                            )
                            nc.tensor.matmul(
                                ps_k[h][:],
                                wk_t[kk][:, ts(h, 128)],
                                xin[:],
                                start=st,
                                stop=sp,
                            )
                        for s in range(4):
                            nc.tensor.matmul(
                                ps_v[s][:],
                                xin[:, ts(s, 128)],
                                wv_t[kk][:],
                                start=st,
                                stop=sp,
                            )
                    # drains split across ACT and DVE, ordered to match the
                    # next tile's matmul emission order (q0,q1,k0,k1,v...)
                    with nc.allow_low_precision(reason="f32r staging copies"):
                        nc.scalar.mul(q_sb[:, 0, ts(n, 512)], ps_q[0][:], SCALE)
                        nc.vector.tensor_scalar_mul(
                            q_sb[:, 1, ts(n, 512)], ps_q[1][:], SCALE
                        )
                        nc.scalar.copy(k_sb[:, 0, ts(n, 512)], ps_k[0][:])
                        nc.vector.tensor_copy(k_sb[:, 1, ts(n, 512)], ps_k[1][:])
                        for s in range(4):
                            if s % 2 == 0:
                                nc.scalar.copy(v_sb[:, 4 * n + s, :], ps_v[s][:])
                            else:
                                nc.vector.tensor_copy(
                                    v_sb[:, 4 * n + s, :], ps_v[s][:]
                                )
                    if b == 0 and n == 0:
                        load_wo_masks()

            # ---- phases 2+3: attention + output projection ----
            with (
                tc.tile_pool(name="ps_s", bufs=2, space="PSUM") as pps,
                tc.tile_pool(name="ps_o", bufs=2, space="PSUM") as ppo,
                tc.tile_pool(name="ps_d", bufs=2, space="PSUM") as ppd,
                tc.tile_pool(name="ps_y", bufs=2, space="PSUM") as ppy,
            ):
                attn_tiles = {}
                pending_tails = []

                def flush_tails():
                    for fn in pending_tails:
                        fn()
                    pending_tails.clear()

                def emit_attention(j, b=b, q_sb=q_sb, k_sb=k_sb, v_sb=v_sb):
                    nblk = 4 * j + 4

                    def win(i):
                        # diagonal block r=i-4j: columns < 128r are fully
                        # masked; keep >=256 moving width for full f32r rate
                        r = i - 4 * j
                        return 128 * min(r, 2) if r >= 0 else 0

                    for hh in (0, 1):
                        po = ppo.tile([128, 512], F32, name="po", tag="po")
                        # 128x128 ones stationary -> every partition of pd
                        # holds the denominator row: the reciprocal and the
                        # po multiply need no cross-partition broadcast
                        pd = ppd.tile([128, 512], F32, name="pd", tag="pd")

                        def emit_s(i):
                            w0 = win(i)
                            psb = pps.tile([128, 512], F32, name="psb", tag="ps")
                            nc.tensor.matmul(
                                psb[:, w0:512],
                                k_sb[:, hh, ds(128 * i, 128)],
                                q_sb[:, hh, ds(512 * j + w0, 512 - w0)],
                                start=True,
                                stop=True,
                            )
                            return psb

                        psb_cur = emit_s(0)
                        for i in range(nblk):
                            psb, psb_cur = psb_cur, (
                                emit_s(i + 1) if i + 1 < nblk else None
                            )
                            w0 = win(i)
                            r = i - 4 * j
                            w_t = wp.tile([128, 512], F32R, name="w_t", tag="w")
                            if r >= 0:  # diagonal block: causal mask
                                # mask*relu(S+b) == relu(mask*(S+b)) for 0/1 mask
                                tmp = wp.tile(
                                    [128, 512], F32, name="wtmp", tag="wtmp"
                                )
                                nc.vector.scalar_tensor_tensor(
                                    tmp[:, w0:512],
                                    psb[:, w0:512],
                                    ATTN_BIAS,
                                    mask_sb[:, r, w0:512],
                                    op0=mybir.AluOpType.add,
                                    op1=mybir.AluOpType.mult,
                                )
                                nc.scalar.activation(
                                    w_t[:, w0:512],
                                    tmp[:, w0:512],
                                    AF.Relu,
                                    bias=0.0,
                                    scale=1.0,
                                )
                            else:
                                nc.scalar.activation(
                                    w_t[:], psb[:], AF.Relu, bias=bias_sb[:], scale=1.0
                                )
                            nc.tensor.matmul(
                                po[:, w0:512],
                                v_sb[:, i, ts(hh, 128)],
                                w_t[:, w0:512],
                                start=i == 0,
                                stop=i == nblk - 1,
                            )
                            nc.tensor.matmul(
                                pd[:, w0:512],
                                ones_sq[:],
                                w_t[:, w0:512],
                                start=i == 0,
                                stop=i == nblk - 1,
                            )
                            if i == 1:
                                # previous head's tail lands here, behind the
                                # current head's first relu, so it never
                                # delays the pipeline warm-up
                                flush_tails()

                        def tail(j=j, hh=hh, po=po, pd=pd):
                            # 1/(den+EPS) as exp(-ln(den+EPS)) on ACT: no
                            # PE broadcast, no single-lane DVE reciprocal
                            lnd = smallp.tile([128, 512], F32, name="lnd", tag="lnd")
                            nc.scalar.activation(
                                lnd[:], pd[:], AF.Ln, bias=eps_sb[:], scale=1.0
                            )
                            rec = smallp.tile([128, 512], F32R, name="rec", tag="rec")
                            with nc.allow_low_precision(
                                reason="f32r normalizer feeds out-proj matmul"
                            ):
                                nc.scalar.activation(
                                    rec[:], lnd[:], AF.Exp, bias=0.0, scale=-1.0
                                )
                            at = attnp.tile(
                                [128, 512], F32R, name=f"at{hh}", tag=f"attn{hh}"
                            )
                            with nc.allow_low_precision(
                                reason="f32r attn staging feeds out-proj matmul"
                            ):
                                nc.vector.tensor_mul(at[:], po[:], rec[:])
                            attn_tiles[(j, hh)] = at

                        pending_tails.append(tail)

                def emit_outproj(j, b=b):
                    a0 = attn_tiles.pop((j, 0))
                    a1 = attn_tiles.pop((j, 1))
                    for s in range(4):
                        yst = ystp.tile([128, C], F16, name="yst", tag="yst")
                        # last tile of the kernel: store per-ot so the final
                        # DMA overlaps the drains instead of trailing them
                        split_dma = b == B - 1 and j == NT - 1 and s == 3
                        for ot in range(4):
                            py = ppy.tile([128, 512], F32, name="py", tag="py")
                            nc.tensor.matmul(
                                py[:],
                                a0[:, ts(s, 128)],
                                wo_sb[:, 0, ts(ot, 512)],
                                start=True,
                                stop=False,
                            )
                            nc.tensor.matmul(
                                py[:],
                                a1[:, ts(s, 128)],
                                wo_sb[:, 1, ts(ot, 512)],
                                start=False,
                                stop=True,
                            )
                            with nc.allow_low_precision(
                                reason="fp16 partial-y staging halves HBM traffic"
                            ):
                                # 1:3 ACT:DVE split - ACT is near-critical
                                # under the late j-groups' relu load
                                if ot == 0:
                                    nc.scalar.copy(yst[:, ts(ot, 512)], py[:])
                                else:
                                    nc.vector.tensor_copy(
                                        yst[:, ts(ot, 512)], py[:]
                                    )
                            if split_dma:
                                nc.gpsimd.dma_start(
                                    y.ap()[
                                        ds(T * b + 512 * j + 128 * s, 128),
                                        ds(512 * ot, 512),
                                    ],
                                    yst[:, ts(ot, 512)],
                                )
                        if not split_dma:
                            nc.gpsimd.dma_start(
                                y.ap()[ds(T * b + 512 * j + 128 * s, 128), :], yst[:]
                            )

                emit_attention(0)
                for j in range(1, NT):
                    emit_attention(j)
                    emit_outproj(j - 1)
                flush_tails()
                emit_outproj(NT - 1)
    if split_waits:
        split_excess_waits(nc)
    return nc


def _host_masks():
    p = np.arange(128, dtype=np.int32)[:, None]
    f = np.arange(512, dtype=np.int32)[None, :]
    return np.stack(
        [(f >= 128 * r + p).astype(np.float32) for r in range(4)], axis=0
    )


def kernel(x, Wq, Wk, Wv, Wo, _trace=False, _trace_kwargs=None):
    global _NC_CACHE
    x = np.ascontiguousarray(np.asarray(x, dtype=np.float32))
    Wq = np.asarray(Wq, dtype=np.float32)
    Wk = np.asarray(Wk, dtype=np.float32)
    Wv = np.asarray(Wv, dtype=np.float32)
    Wo = np.asarray(Wo, dtype=np.float32)

    if _NC_CACHE is None:
        _NC_CACHE = _build()
    nc = _NC_CACHE

    xT = np.ascontiguousarray(x.reshape(B * T, C).T)
    masks = _host_masks()
    in_maps = []
    for c in range(N_CORES):
        sl = slice(CH * c, CH * (c + 1))
        in_maps.append(
            {
                "xT": xT,
                "wq": np.ascontiguousarray(Wq[sl, :].T),
                "wk": np.ascontiguousarray(Wk[sl, :].T),
                "wv": np.ascontiguousarray(Wv[sl, :].T),
                "wo": np.ascontiguousarray(Wo[:, sl].T),
                "masks": masks,
            }
        )

    res = run_bass_kernel_spmd(
        nc,
        in_maps,
        core_ids=list(range(N_CORES)),
        trace=_trace,
        **(_trace_kwargs or {}),
    )
    acc = np.zeros((B * T, C), dtype=np.float64)
    for c in range(N_CORES):
        acc += res.results[c]["y"].astype(np.float64)
    out = acc.astype(np.float32).reshape(B, T, C)
    if _trace:
        return out, res
    return out
